# revision 4
# baseline (speedup 1.0000x reference)
"""Trainium2 kernel v2 for nn_Custom_Model_Embedding_Bag_Sum.

Math (same as baseline): out[j] = sum_v c[v] * A[j, v] with c = histogram of
eb_input over the 1M vocab and A the 26 "planes" derived from W on the host:
tables 0-4,7-9 contribute their 3 dims as separate planes; tables 5 and 6 are
pre-summed over dim (the reference folds them to [B,1]) so they contribute one
plane each -> 26 planes instead of 30 (13% less traffic and compute).

Distribution: vocab-sharded across 8 cores; each core gets
  w: [128, 26*Q] f16  (plane-major; partition p, free = (plane j, q))
  c: [128, Q]   f16   (histogram shard)
and computes red[p, j] = sum_q w[p, jQ+q] * c[p, q].

Engine split (the v1 bottleneck was 30 scalar_tensor_tensor ops, which only
run at 1x on the DVE = 1079ns each):
  - K "fused" planes:   DVE scalar_tensor_tensor (1x, 1079ns) -> red directly
  - 26-K "split" planes: DVE tensor_tensor mult (2x, 570ns) -> prod ring,
                         ACT activation(Identity, accum_out) (1000ns) -> red
DVE time = K*1079 + (26-K)*570, ACT time = (26-K)*1000; K=7 balances both at
~18.6us, matching the f16 DMA floor (6.75MB @ ~358GB/s = 18.9us). All three
engines run concurrently; the kernel is at the joint roofline.

Final: PE matmul(ones, red) reduces partitions -> [1, 26] f32, DMA'd out.
Host sums the 8 cores' [128->1, 26] outputs and assembles the 26-vector.
"""

import os
import sys

import numpy as np

if "/opt/trn_rl_repo" not in sys.path:
    sys.path.insert(0, "/opt/trn_rl_repo")

NUM_TABLES = 10
EMB_DIM = 3
VOCAB = 1_000_000
N_CORES = 8
P = 128
Q = 978                      # free elems per partition per plane (even, 4B-aligned f16)
V_CORE = P * Q               # 125184 vocab rows per core
N_PLANES = 26                # 8 tables x 3 dims + 2 folded tables

DT = "float16"

# knobs
K_FUSED = int(os.environ.get("EBAG_KFUSED", "9"))   # planes done fully on DVE via stt
N_PROD = int(os.environ.get("EBAG_NPROD", "6"))     # prod ring slots
# DMA group plan: plane counts per dma_start (first groups small to cut ramp)
GROUPS = tuple(
    int(x) for x in os.environ.get("EBAG_GROUPS", "1,1,2,3,3,4,4,4,4").split(",")
)
assert sum(GROUPS) == N_PLANES

_COMPILED = {}


def _plane_sets():
    """Assign planes to fused(stt)/split(TT+ACT) sets, interleaved so ACT is
    fed from the start and all engines drain together."""
    fused = set()
    if K_FUSED > 0:
        stride = N_PLANES / K_FUSED
        fused = {min(N_PLANES - 1, int((i + 0.7) * stride)) for i in range(K_FUSED)}
        while len(fused) < K_FUSED:  # collision fallback
            fused.add(max(0, min(N_PLANES - 1, len(fused))))
    return fused


def _build_nc(reps=1):
    import concourse.bass as bass
    from concourse import mybir

    dt = mybir.dt.float16
    f32 = mybir.dt.float32
    A = mybir.AluOpType
    AF = mybir.ActivationFunctionType

    fused = _plane_sets()
    split_planes = [i for i in range(N_PLANES) if i not in fused]
    ng = len(GROUPS)
    # plane -> dma group index
    plane_group = []
    for g, cnt in enumerate(GROUPS):
        plane_group += [g] * cnt

    nc = bass.Bass()
    w = nc.dram_tensor("w", [P, N_PLANES * Q], dt, kind="ExternalInput")
    c = nc.dram_tensor("c", [P, Q], dt, kind="ExternalInput")
    o = nc.dram_tensor("o", [1, N_PLANES], f32, kind="ExternalOutput")

    with (
        nc.sbuf_tensor([P, N_PLANES * Q], dt) as w_sb,
        nc.sbuf_tensor([P, Q], dt) as c_sb,
        nc.sbuf_tensor([P, N_PROD * Q], dt) as prod,   # ring for split planes
        nc.sbuf_tensor([P, Q], dt) as dump,            # stt mandatory out
        nc.sbuf_tensor([P, N_PLANES], f32) as red,
        nc.sbuf_tensor([P, 1], f32) as ones,
        nc.sbuf_tensor([1, N_PLANES], f32) as out_sb,
        nc.psum_tensor([1, N_PLANES], f32) as acc,
        nc.semaphore() as dma_sem,    # sync -> compute: 16 per dma
        nc.semaphore() as prog_sem,   # DVE: +1 after every plane op
        nc.semaphore() as a2v_sem,    # ACT: +1 after every reduce
        nc.semaphore() as pe_sem,     # PE -> DVE copy -> sync out-dma
        nc.Block() as block,
    ):
        n_split = len(split_planes)
        group_last = [sum(GROUPS[: g + 1]) - 1 for g in range(ng)]
        # global plane index of split plane k (for ACT waits on prog_sem)
        split_pos = {k: i for k, i in enumerate(split_planes)}

        @block.sync
        def _(sync):
            sync.dma_start(c_sb[:], c[:]).then_inc(dma_sem, 16)
            for r in range(reps):
                for g in range(ng):
                    if r > 0:
                        # WAR: DVE must be done with this group's planes from
                        # the previous rep before we overwrite them.
                        sync.wait_ge(prog_sem, (r - 1) * N_PLANES + group_last[g] + 1)
                    lo = sum(GROUPS[:g])
                    sl = slice(lo * Q, (lo + GROUPS[g]) * Q)
                    sync.dma_start(w_sb[:, sl], w[:, sl]).then_inc(dma_sem, 16)
            sync.wait_ge(pe_sem, 2)
            sync.dma_start(o[:], out_sb[:]).then_inc(dma_sem, 16)
            sync.wait_ge(dma_sem, 16 * (reps * ng + 2))

        @block.vector
        def _(vector):
            vector.memset(ones[:], 1.0)
            for r in range(reps):
                tt_k = r * n_split  # global TT (split-plane) index
                for i in range(N_PLANES):
                    g = plane_group[i]
                    vector.wait_ge(dma_sem, 16 * (r * ng + g + 2))
                    if i in fused:
                        vector.scalar_tensor_tensor(
                            dump[:],
                            w_sb[:, i * Q:(i + 1) * Q],
                            1.0,
                            c_sb[:],
                            op0=A.mult,
                            op1=A.mult,
                            accum_out=red[:, i:i + 1],
                        ).then_inc(prog_sem)
                    else:
                        s = tt_k % N_PROD
                        if tt_k >= N_PROD:
                            # WAR: ACT must have consumed this slot
                            vector.wait_ge(a2v_sem, tt_k - N_PROD + 1)
                        vector.tensor_tensor(
                            prod[:, s * Q:(s + 1) * Q],
                            w_sb[:, i * Q:(i + 1) * Q],
                            c_sb[:],
                            op=A.mult,
                        ).then_inc(prog_sem)
                        tt_k += 1
            vector.wait_ge(pe_sem, 1)
            vector.tensor_copy(out_sb[:], acc[:]).then_inc(pe_sem)

        @block.scalar
        def _(scalar):
            for r in range(reps):
                for k, i in enumerate(split_planes):
                    tt_k = r * n_split + k
                    s = tt_k % N_PROD
                    scalar.wait_ge(prog_sem, r * N_PLANES + i + 1)
                    scalar.activation(
                        prod[:, s * Q:(s + 1) * Q],
                        prod[:, s * Q:(s + 1) * Q],
                        AF.Identity,
                        accum_out=red[:, i:i + 1],
                    ).then_inc(a2v_sem)

        @block.tensor
        def _(tensor):
            tensor.wait_ge(prog_sem, reps * N_PLANES)
            if n_split:
                tensor.wait_ge(a2v_sem, reps * n_split)
            tensor.matmul(acc[:], ones[:], red[:], start=True, stop=True).then_inc(
                pe_sem
            )

    sems = (dma_sem, prog_sem, a2v_sem, pe_sem)
    sem_lo = min(s.num for s in sems)
    sem_hi = max(s.num for s in sems)
    nc.sync.drain(semaphore_range=range(sem_lo, sem_hi + 1))
    for s in sems:
        nc.sync.sem_clear(s)
    return nc


def _get_nc(np_dt=None, pg=None, reps=1):
    key = reps
    if key not in _COMPILED:
        _COMPILED[key] = _build_nc(reps)
    return _COMPILED[key]


def _auto_pg(np_dt=None):
    return 0


def _prep_inputs(eb_input, W, np_dt=None):
    """Per-core input maps: histogram shard + swizzled 26-plane shard."""
    np_dt = np.float16
    counts = np.bincount(eb_input.astype(np.int64), minlength=VOCAB)
    counts_pad = np.zeros(N_CORES * V_CORE, dtype=np_dt)
    counts_pad[:VOCAB] = counts.astype(np_dt)
    c_sh = counts_pad.reshape(N_CORES, P, Q)

    # Build the 26 planes: [26, VOCAB] f32 view of W (tables 5,6 dim-summed)
    planes = np.empty((N_PLANES, VOCAB), dtype=np.float32)
    j = 0
    for t in range(NUM_TABLES):
        if t in (5, 6):
            planes[j] = W[t].sum(axis=1)
            j += 1
        else:
            for d in range(EMB_DIM):
                planes[j] = W[t, :, d]
                j += 1
    assert j == N_PLANES

    in_maps = []
    for k in range(N_CORES):
        v0, v1 = k * V_CORE, (k + 1) * V_CORE
        if v1 <= VOCAB:
            pk = planes[:, v0:v1]
        else:
            pk = np.zeros((N_PLANES, V_CORE), dtype=np.float32)
            pk[:, :VOCAB - v0] = planes[:, v0:]
        # [26, V_CORE] -> [26, 128, Q] -> (p, j, q) -> [128, 26*Q]
        wk = np.ascontiguousarray(
            pk.reshape(N_PLANES, P, Q).transpose(1, 0, 2), dtype=np_dt
        ).reshape(P, N_PLANES * Q)
        in_maps.append({"w": wk, "c": np.ascontiguousarray(c_sh[k])})
    return in_maps


def _assemble(partials):
    """partials: [n_cores, 1, 26] (or [n_cores, 26]) f32 -> final [26]."""
    S = partials.reshape(N_CORES, -1, N_PLANES).sum(axis=(0, 1))
    # plane order is already the output order: t0d0..t4d2, t5sum, t6sum, t7d0..t9d2
    return S.astype(np.float32)


def kernel(eb_input, eb_offset, W):
    from concourse.bass_utils import run_bass_kernel_spmd

    nc = _get_nc(reps=1)
    in_maps = _prep_inputs(np.asarray(eb_input), np.asarray(W))
    res = run_bass_kernel_spmd(nc, in_maps, core_ids=list(range(N_CORES)))
    partials = np.stack([r["o"] for r in res.results])
    return _assemble(partials)
